# revision 1
# baseline (speedup 1.0000x reference)
"""MedianConvolution (gnn message passing) — Trainium2 Bass kernel, 8 cores.

Computes: h = x @ kernel; msg = h[neighbors]; out = exact midpoint median
over the K=32 neighbor axis (ranks 15,16 of the sort), i.e.
tfp percentile(q=50, interpolation='midpoint').

Distribution: nodes (rows of x's output / neighbors) are sharded across the
8 NeuronCores; every core computes the full h = x @ kernel on-device
(kernel/x replicated) and gathers only its own node shard's neighbor rows.

Per-core SPMD program:
  phase 1  GEMM: xT is supplied host-pre-transposed ([256, N]); PE computes
           h tile-by-tile (fp32, PSUM-accumulated over the two 128-feature
           chunks); h rows are written to DRAM, split into two halves
           (h_lo = rows [0, N/2), h_hi = rows [N/2, N)), each with one extra
           "+BIG" dummy row. The split exists because dma_gather indices are
           int16 (max 32767 < N); every neighbor is fetched by two gather
           calls (one per half, the miss side pointing at the dummy row) and
           merged with one elementwise min.
  phase 2  per chunk of C shard nodes: gpsimd.dma_gather pulls the 256-byte
           h rows for all 32 neighbor planes (k-major layout
           [128, K, C/128, 64]), a TT-min merges the lo/hi candidates, a
           Batcher odd-even mergesort sorts planes 0-15 and 16-31 (strided
           multi-dim APs, ping-pong between two buffers, untouched planes
           copied on the Scalar engine), and the 32-way median pair is
           extracted with the anti-diagonal identity
              low = max_i min(A_i, B_15-i),  up = min_i max(A_i, B_15-i)
           via two TT ops + two segmented tensor_reduce ops. The midpoint
           (low+up)/2 is DMAed out.
"""
from contextlib import ExitStack

import numpy as np

import concourse.bass as bass
import concourse.tile as tile
from concourse import bacc, bass_utils, library_config, mybir
from concourse.tile_rust import add_dep_helper

F32 = mybir.dt.float32
I16 = mybir.dt.int16
P = 128
U = 64  # units
K = 32  # neighbors
FEAT = 256
N_NODES = 50000
BIG = 1.0e30
NUM_CORES = 8
CHUNK = 256  # shard nodes per chunk
NET_BUFS = 4

# Batcher odd-even mergesort(16) stages; verified against np.sort via the
# 0-1 principle. Each stage: comparators (k, k+d) for k = i*f + r over the
# slices below, applied to both 16-plane halves. cp = untouched plane
# slices (copied to the ping-pong destination).
SORT16_STAGES = [
    dict(f=2, i=(0, 8, 1), r=(0, 1, 1), d=1, cp=[]),
    dict(f=4, i=(0, 4, 1), r=(0, 2, 1), d=2, cp=[]),
    dict(f=4, i=(0, 4, 1), r=(1, 2, 1), d=1, cp=[(0, 16, 4), (3, 16, 4)]),
    dict(f=8, i=(0, 2, 1), r=(0, 4, 1), d=4, cp=[]),
    dict(f=8, i=(0, 2, 1), r=(2, 4, 1), d=2,
         cp=[(0, 16, 8), (1, 16, 8), (6, 16, 8), (7, 16, 8)]),
    dict(f=8, i=(0, 2, 1), r=(1, 6, 2), d=1, cp=[(0, 16, 8), (7, 16, 8)]),
    dict(f=16, i=(0, 1, 1), r=(0, 8, 1), d=8, cp=[]),
    dict(f=16, i=(0, 1, 1), r=(4, 8, 1), d=4, cp=[(0, 4, 1), (12, 16, 1)]),
    dict(f=4, i=(0, 3, 1), r=(2, 4, 1), d=2, cp=[(0, 2, 1), (14, 16, 1)]),
    dict(f=2, i=(0, 7, 1), r=(1, 2, 1), d=1, cp=[(0, 16, 15)]),
]


def build_kernel(nrows, shard_nodes, C, num_cores=NUM_CORES, gemm_super=2048,
                 net_bufs=NET_BUFS):
    assert nrows % 2 == 0
    HALF = nrows // 2
    NCHUNK = shard_nodes // C
    assert NCHUNK * C == shard_nodes
    B = C // P
    NIDX = C * K
    IDXCOLS = NIDX // 16

    nc = bacc.Bacc(
        "TRN2",
        target_bir_lowering=False,
        debug=False,
        num_devices=num_cores,
    )

    xT = nc.dram_tensor("xT", [FEAT, nrows], F32, kind="ExternalInput").ap()
    wk = nc.dram_tensor("wk", [FEAT, U], F32, kind="ExternalInput").ap()
    idx = nc.dram_tensor("idx", [NCHUNK, 2, P, IDXCOLS], I16, kind="ExternalInput").ap()
    out = nc.dram_tensor("out", [NCHUNK, P, B * U], F32, kind="ExternalOutput").ap()
    h_lo = nc.dram_tensor("h_lo", [HALF + 1, U], F32, kind="Internal").ap()
    h_hi = nc.dram_tensor("h_hi", [HALF + 1, U], F32, kind="Internal").ap()

    with tile.TileContext(nc) as tc:
        with ExitStack() as ctx:
            # ---------------- phase 1: GEMM ----------------
            ctx1 = ctx.enter_context(ExitStack())
            g_x = ctx1.enter_context(tc.tile_pool(name="g_x", bufs=2))
            g_w = ctx1.enter_context(tc.tile_pool(name="g_w", bufs=1))
            g_h = ctx1.enter_context(tc.tile_pool(name="g_h", bufs=2))
            g_ps = ctx1.enter_context(tc.tile_pool(name="g_ps", bufs=4, space="PSUM"))

            wkt = g_w.tile([P, 2 * U], F32)
            nc.sync.dma_start(wkt[:, 0:U], wk[0:P, :])
            nc.sync.dma_start(wkt[:, U : 2 * U], wk[P : 2 * P, :])

            h_writes = []
            dummy = g_w.tile([1, U], F32)
            nc.vector.memset(dummy[:], BIG)
            h_writes.append(nc.sync.dma_start(h_lo[HALF : HALF + 1, :], dummy[:]))
            h_writes.append(nc.sync.dma_start(h_hi[HALF : HALF + 1, :], dummy[:]))

            S = gemm_super
            n_super = (nrows + S - 1) // S
            for s in range(n_super):
                n0 = s * S
                ncnt = min(S, nrows - n0)
                ntiles = (ncnt + P - 1) // P
                xt0 = g_x.tile([P, S], F32, tag="xt0")
                xt1 = g_x.tile([P, S], F32, tag="xt1")
                nc.sync.dma_start(xt0[:, 0:ncnt], xT[0:P, n0 : n0 + ncnt])
                nc.sync.dma_start(xt1[:, 0:ncnt], xT[P : 2 * P, n0 : n0 + ncnt])
                hb = g_h.tile([P, (S // P) * U], F32, tag="hb")
                for t in range(ntiles):
                    c0 = t * P
                    cw = min(P, ncnt - c0)
                    ps = g_ps.tile([P, U], F32)
                    nc.tensor.matmul(
                        ps[0:cw, :], xt0[:, c0 : c0 + cw], wkt[:, 0:U],
                        start=True, stop=False,
                    )
                    nc.tensor.matmul(
                        ps[0:cw, :], xt1[:, c0 : c0 + cw], wkt[:, U : 2 * U],
                        start=False, stop=True,
                    )
                    nc.scalar.copy(hb[0:cw, t * U : (t + 1) * U], ps[0:cw, :])
                hb3 = hb[:].rearrange("p (t u) -> p t u", u=U)
                # write h rows into the lo/hi half regions (straddle-aware,
                # full 128-row tiles coalesced into single DMAs)
                for lim0, lim1, dst, base in (
                    (n0, min(n0 + ncnt, HALF), h_lo, 0),
                    (max(n0, HALF), n0 + ncnt, h_hi, HALF),
                ):
                    if lim1 <= lim0:
                        continue
                    ta = (lim0 - n0 + P - 1) // P
                    tb = (lim1 - n0) // P
                    segs = []
                    if ta > tb:
                        segs.append((lim0, lim1))
                    else:
                        if lim0 < n0 + ta * P:
                            segs.append((lim0, n0 + ta * P))
                        if tb > ta:
                            segs.append((n0 + ta * P, n0 + tb * P))
                        if n0 + tb * P < lim1:
                            segs.append((n0 + tb * P, lim1))
                    for r0, r1 in segs:
                        nt = (r1 - r0) // P
                        if nt >= 1 and (r0 - n0) % P == 0:
                            tt = (r0 - n0) // P
                            dr = dst[r0 - base : r1 - base, :].rearrange(
                                "(o p) u -> p o u", p=P
                            )
                            srcv = hb3[:, tt : tt + nt, :]
                        else:
                            tt = (r0 - n0) // P
                            p0 = r0 - (n0 + tt * P)
                            p1 = r1 - (n0 + tt * P)
                            dr = dst[r0 - base : r1 - base, :].rearrange(
                                "(o p) u -> p o u", p=p1 - p0
                            )
                            srcv = hb3[p0:p1, tt : tt + 1, :]
                        h_writes.append(nc.sync.dma_start(dr, srcv))

            # ---------------- phase 2: gather + median ----------------
            ctx1.close()
            g_net = ctx.enter_context(tc.tile_pool(name="g_net", bufs=net_bufs))
            g_idx = ctx.enter_context(tc.tile_pool(name="g_idx", bufs=2))
            g_out = ctx.enter_context(tc.tile_pool(name="g_out", bufs=2))
            g_big = ctx.enter_context(tc.tile_pool(name="g_big", bufs=1))

            nc.gpsimd.load_library(library_config.mlp)
            med_all = g_big.tile([P, NCHUNK * B * U], F32, tag="medall")
            n_g = 0
            BU = B * U
            # per-call index count capped by the 128-entry SWDGE ring
            KG = max(1, 1920 // C)
            kgroups = []
            k0 = 0
            while k0 < K:
                kgroups.append((k0, min(K, k0 + KG)))
                k0 += KG

            for c in range(NCHUNK):
                ia = g_idx.tile([P, IDXCOLS], I16, tag="ia")
                ib = g_idx.tile([P, IDXCOLS], I16, tag="ib")
                nc.sync.dma_start(ia[:], idx[c, 0])
                nc.sync.dma_start(ib[:], idx[c, 1])
                ra = g_net.tile([P, K * BU], F32, tag="ra")
                rb = g_net.tile([P, K * BU], F32, tag="rb")
                for reg, it, hsrc in ((ra, ia, h_lo), (rb, ib, h_hi)):
                    for ka, kb in kgroups:
                        nidx = C * (kb - ka)
                        g = nc.gpsimd.dma_gather(
                            reg[:, ka * BU : kb * BU].rearrange("p (j e) -> p j e", e=U),
                            hsrc[:],
                            it[:, ka * C // 16 : kb * C // 16],
                            nidx,
                            nidx,
                            U,
                            single_packet=False,
                        )
                        if n_g == 0:
                            for w in h_writes:
                                add_dep_helper(
                                    g.ins, w.ins,
                                    reason="gather waits for h DRAM writes",
                                )
                        n_g += 1
                # merge lo/hi candidates (dummy rows are +BIG)
                nc.vector.tensor_tensor(
                    out=ra[:], in0=ra[:], in1=rb[:], op=mybir.AluOpType.min
                )

                # Batcher network over both halves, ping-pong ra <-> rb
                src, dst = ra, rb
                for sp in SORT16_STAGES:
                    f = sp["f"]
                    ni = 16 // f
                    i_full = sp["i"] == (0, ni, 1)
                    d = sp["d"]
                    di, dr = d // f, d % f
                    r_vals = list(range(*sp["r"]))
                    if r_vals[-1] + dr >= f:
                        assert all(rv + dr >= f for rv in r_vals), sp
                        di, dr = di + 1, dr - f
                    r_sl = slice(*sp["r"])
                    hi_r = slice(sp["r"][0] + dr, sp["r"][1] + dr, sp["r"][2])
                    if i_full and di == 0:
                        vs = src[:].rearrange("p (hi r bu) -> p hi r bu", r=f, bu=BU)
                        vd = dst[:].rearrange("p (hi r bu) -> p hi r bu", r=f, bu=BU)
                        lo_s = vs[:, :, r_sl, :]
                        hi_s = vs[:, :, hi_r, :]
                        nc.vector.tensor_tensor(
                            out=vd[:, :, r_sl, :], in0=lo_s, in1=hi_s,
                            op=mybir.AluOpType.min,
                        )
                        nc.vector.tensor_tensor(
                            out=vd[:, :, hi_r, :], in0=lo_s, in1=hi_s,
                            op=mybir.AluOpType.max,
                        )
                    else:
                        i_sl = slice(*sp["i"])
                        hi_i = slice(sp["i"][0] + di, sp["i"][1] + di, sp["i"][2])
                        vs = src[:].rearrange(
                            "p (hh i r bu) -> p hh i r bu", hh=2, i=ni, r=f, bu=BU
                        )
                        vd = dst[:].rearrange(
                            "p (hh i r bu) -> p hh i r bu", hh=2, i=ni, r=f, bu=BU
                        )
                        lo_s = vs[:, :, i_sl, r_sl, :]
                        hi_s = vs[:, :, hi_i, hi_r, :]
                        nc.vector.tensor_tensor(
                            out=vd[:, :, i_sl, r_sl, :], in0=lo_s, in1=hi_s,
                            op=mybir.AluOpType.min,
                        )
                        nc.vector.tensor_tensor(
                            out=vd[:, :, hi_i, hi_r, :], in0=lo_s, in1=hi_s,
                            op=mybir.AluOpType.max,
                        )
                    vks = src[:].rearrange("p (hh kk bu) -> p hh kk bu", hh=2, kk=16)
                    vkd = dst[:].rearrange("p (hh kk bu) -> p hh kk bu", hh=2, kk=16)
                    for cpsl in sp["cp"]:
                        ks = slice(*cpsl)
                        nc.scalar.copy(vkd[:, :, ks, :], vks[:, :, ks, :])
                    src, dst = dst, src

                # anti-diagonal merge of the two sorted 16-plane halves
                vk = src[:].rearrange("p (k bu) -> p k bu", k=K)
                vo = dst[:].rearrange("p (k bu) -> p k bu", k=K)
                A = vk[:, 0:16, :]
                Brev = vk[:, 31:15:-1, :]
                nc.vector.tensor_tensor(
                    out=vo[:, 0:16, :], in0=A, in1=Brev, op=mybir.AluOpType.max
                )
                nc.vector.tensor_tensor(
                    out=vk[:, 0:16, :], in0=A, in1=Brev, op=mybir.AluOpType.min
                )
                low = g_out.tile([P, BU], F32, tag="low")
                up = g_out.tile([P, BU], F32, tag="up")
                src_r = src[:].rearrange("p (k bu) -> p bu k", k=K)[:, :, 0:16]
                dst_r = dst[:].rearrange("p (k bu) -> p bu k", k=K)[:, :, 0:16]
                nc.vector.tensor_reduce(
                    out=low[:], in_=src_r, axis=mybir.AxisListType.X,
                    op=mybir.AluOpType.max,
                )
                nc.vector.tensor_reduce(
                    out=up[:], in_=dst_r, axis=mybir.AxisListType.X,
                    op=mybir.AluOpType.min,
                )
                ms = med_all[:, c * BU : (c + 1) * BU]
                nc.vector.tensor_tensor(
                    out=ms, in0=low[:], in1=up[:], op=mybir.AluOpType.add
                )
                nc.scalar.mul(ms, ms, 0.5)
                nc.sync.dma_start(out[c], ms)

    nc.compile()
    return nc


def _prep_inputs(x, neighbors, kern, num_cores=NUM_CORES, C=CHUNK):
    nrows = x.shape[0]
    HALF = nrows // 2
    total = neighbors.shape[0]
    shard = (total + num_cores - 1) // num_cores
    NCHUNK = (shard + C - 1) // C
    shard_pad = NCHUNK * C
    B = C // P
    NIDX = C * K
    IDXCOLS = NIDX // 16

    xT = np.ascontiguousarray(x.T).astype(np.float32, copy=False)
    wk = np.ascontiguousarray(kern).astype(np.float32, copy=False)

    in_maps = []
    for core in range(num_cores):
        n0 = core * shard
        nbr = np.full((shard_pad, K), nrows, dtype=np.int64)
        real = min(shard, total - n0)
        nbr[:real] = neighbors[n0 : n0 + real]
        idxarr = np.empty((NCHUNK, 2, P, IDXCOLS), dtype=np.int16)
        for c in range(NCHUNK):
            nb3 = nbr[c * C : (c + 1) * C].reshape(B, P, K)
            v = nb3.transpose(2, 0, 1).reshape(-1)  # i = ((k*B + b)*128 + p)
            lo = np.where(v < HALF, v, HALF).astype(np.int16)
            hi = np.where(v >= HALF, v - HALF, HALF).astype(np.int16)
            for j, arr in ((0, lo), (1, hi)):
                # logical index i lives at [i%16, i//16]; replicated to all
                # eight 16-partition groups (Q7 core pairs read their own)
                idxarr[c, j] = np.tile(arr.reshape(IDXCOLS, 16).T, (P // 16, 1))
        in_maps.append({"xT": xT, "wk": wk, "idx": idxarr})
    meta = dict(shard=shard, shard_pad=shard_pad, NCHUNK=NCHUNK, C=C, total=total)
    return in_maps, meta


def _unshard_output(results, meta, num_cores=NUM_CORES):
    outs = []
    for core in range(num_cores):
        o = results[core]["out"]  # [NCHUNK, P, B*U]
        NCHUNK, _, BU = o.shape
        B = BU // U
        o = (
            o.reshape(NCHUNK, P, B, U)
            .transpose(0, 2, 1, 3)
            .reshape(meta["shard_pad"], U)
        )
        outs.append(o[: meta["shard"]])
    return np.concatenate(outs, axis=0)[: meta["total"]]


_CACHE = {}


def kernel(x, neighbors, kernel):
    """Full inputs in, full output out. Shards nodes across 8 NeuronCores."""
    x = np.asarray(x, dtype=np.float32)
    neighbors_np = np.asarray(neighbors)
    kern = np.asarray(kernel, dtype=np.float32)
    assert x.shape[1] == FEAT and kern.shape == (FEAT, U)
    assert neighbors_np.shape[1] == K

    in_maps, meta = _prep_inputs(x, neighbors_np, kern)
    key = (x.shape[0], meta["shard_pad"], meta["C"])
    if key not in _CACHE:
        _CACHE[key] = build_kernel(x.shape[0], meta["shard_pad"], meta["C"])
    nc = _CACHE[key]
    res = bass_utils.run_bass_kernel_spmd(
        nc, in_maps, core_ids=list(range(NUM_CORES))
    )
    return _unshard_output(res.results, meta)



# revision 9
# speedup vs baseline: 1.7899x; 1.7899x over previous
"""MedianConvolution (gnn message passing) — Trainium2 Bass kernel, 8 cores.

Computes: h = x @ kernel; msg = h[neighbors]; out = exact midpoint median
over the K=32 neighbor axis (ranks 15,16 of the sort), i.e.
tfp percentile(q=50, interpolation='midpoint').

Distribution: nodes (rows of neighbors) are sharded across the 8 NeuronCores;
every core computes the full h = x @ kernel on-device (x/kernel replicated,
fp16 inputs, fp32 PSUM accumulate) and gathers only its own node shard's
neighbor rows.

Key layout trick: h is stored in DRAM as PAIRS h_pair[r] = [h[2r] | h[2r+1]]
(fp16, 256B rows). dma_gather requires 256B-aligned elements and int16
indices; the pair layout satisfies both (idx = node>>1 <= 24999) with ONE
descriptor per neighbor instead of the two (lo/hi halves) the fp32 baseline
needed. The wrong pair half is discarded on-chip with a single
copy_predicated keyed on a host-uploaded parity mask.

The whole median datapath runs in fp16: TensorTensor min/max supports the
DVE 2x_1p fast mode (2-byte dtypes) for 2x throughput, and the final
midpoint is exact up to fp16 rounding (~0.05%), far inside the 2e-2 gate.
The x0.5 of the midpoint is folded into the GEMM weights (median is
scale-equivariant for positive scales).

Per-core SPMD program:
  phase 1  GEMM: xT fp16 [256, N] x wk fp16 [256, 64] -> PSUM fp32, copied
           to fp16 and DMAed into the pair layout.
  phase 2  per chunk of C=256 shard nodes: one 8192-index dma_gather pulls
           the 256B pair rows for all 32 neighbor planes (k-major), a
           copy_predicated resolves pair parity in place, a Batcher
           odd-even mergesort sorts planes 0-15 and 16-31 (fp16 TT min/max,
           untouched planes copied on the Scalar engine), the 32-way median
           pair comes from the anti-diagonal identity
              low = max_i min(A_i, B_15-i),  up = min_i max(A_i, B_15-i)
           via two TT ops + two min/max trees, and low+up (already scaled
           by 0.5) is written out in fp32.
"""
from contextlib import ExitStack

import numpy as np

import concourse.bass as bass
import concourse.tile as tile
from concourse import bacc, bass_utils, library_config, mybir
from concourse.tile_rust import add_dep_helper

F32 = mybir.dt.float32
F16 = mybir.dt.float16
I16 = mybir.dt.int16
U8 = mybir.dt.uint8
P = 128
U = 64  # units
K = 32  # neighbors
FEAT = 256
N_NODES = 50000
NUM_CORES = 8
CHUNK = 256  # shard nodes per chunk
NET_BUFS = 4
GATHER_SPLIT = 1  # dma_gather calls per chunk

# Batcher odd-even mergesort(16) stages; verified against np.sort via the
# 0-1 principle. Each stage: comparators (k, k+d) for k = i*f + r over the
# slices below, applied to both 16-plane halves. cp = untouched plane
# slices (copied to the ping-pong destination).
SORT16_STAGES = [
    dict(f=2, i=(0, 8, 1), r=(0, 1, 1), d=1, cp=[]),
    dict(f=4, i=(0, 4, 1), r=(0, 2, 1), d=2, cp=[]),
    dict(f=4, i=(0, 4, 1), r=(1, 2, 1), d=1, cp=[(0, 16, 4), (3, 16, 4)]),
    dict(f=8, i=(0, 2, 1), r=(0, 4, 1), d=4, cp=[]),
    dict(f=8, i=(0, 2, 1), r=(2, 4, 1), d=2,
         cp=[(0, 16, 8), (1, 16, 8), (6, 16, 8), (7, 16, 8)]),
    dict(f=8, i=(0, 2, 1), r=(1, 6, 2), d=1, cp=[(0, 16, 8), (7, 16, 8)]),
    dict(f=16, i=(0, 1, 1), r=(0, 8, 1), d=8, cp=[]),
    dict(f=16, i=(0, 1, 1), r=(4, 8, 1), d=4, cp=[(0, 4, 1), (12, 16, 1)]),
    dict(f=4, i=(0, 3, 1), r=(2, 4, 1), d=2, cp=[(0, 2, 1), (14, 16, 1)]),
    dict(f=2, i=(0, 7, 1), r=(1, 2, 1), d=1, cp=[(0, 16, 15)]),
]


def build_kernel(nrows, shard_nodes, C, num_cores=NUM_CORES, gemm_super=2048,
                 net_bufs=NET_BUFS, gather_split=GATHER_SPLIT):
    nrows = ((nrows + P - 1) // P) * P  # host pads xT to the same size
    NPAIR = nrows // 2
    NCHUNK = shard_nodes // C
    assert NCHUNK * C == shard_nodes
    B = C // P
    BU = B * U
    NIDX = C * K          # gather indices (pair rows) per chunk
    IDXCOLS = NIDX // 16
    J = NIDX // P         # gathered columns per chunk (= K * B)

    nc = bacc.Bacc(
        "TRN2",
        target_bir_lowering=False,
        debug=False,
        num_devices=num_cores,
    )

    xT = nc.dram_tensor("xT", [FEAT, nrows], F16, kind="ExternalInput").ap()
    wk = nc.dram_tensor("wk", [FEAT, U], F16, kind="ExternalInput").ap()
    idx = nc.dram_tensor("idx", [NCHUNK, P, IDXCOLS], I16, kind="ExternalInput").ap()
    par = nc.dram_tensor("par", [NCHUNK, P, J], U8, kind="ExternalInput").ap()
    out = nc.dram_tensor("out", [NCHUNK, P, BU], F32, kind="ExternalOutput").ap()
    h_pair = nc.dram_tensor("h_pair", [NPAIR, 2 * U], F16, kind="Internal").ap()

    with tile.TileContext(nc) as tc:
        with ExitStack() as ctx:
            # ---------------- phase 1: GEMM ----------------
            ctx1 = ctx.enter_context(ExitStack())
            g_x = ctx1.enter_context(tc.tile_pool(name="g_x", bufs=2))
            g_w = ctx1.enter_context(tc.tile_pool(name="g_w", bufs=1))
            g_h = ctx1.enter_context(tc.tile_pool(name="g_h", bufs=2))
            g_ps = ctx1.enter_context(tc.tile_pool(name="g_ps", bufs=4, space="PSUM"))

            wkt = g_w.tile([P, 2 * U], F16)
            nc.sync.dma_start(wkt[:, 0:U], wk[0:P, :])
            nc.sync.dma_start(wkt[:, U : 2 * U], wk[P : 2 * P, :])

            h_writes = []
            S = gemm_super
            n_super = (nrows + S - 1) // S
            for s in range(n_super):
                n0 = s * S
                ncnt = min(S, nrows - n0)
                assert ncnt % P == 0
                ntiles = ncnt // P
                xt0 = g_x.tile([P, S], F16, tag="xt0")
                xt1 = g_x.tile([P, S], F16, tag="xt1")
                nc.sync.dma_start(xt0[:, 0:ncnt], xT[0:P, n0 : n0 + ncnt])
                nc.sync.dma_start(xt1[:, 0:ncnt], xT[P : 2 * P, n0 : n0 + ncnt])
                hb = g_h.tile([P, (S // P) * U], F16, tag="hb")
                for t in range(ntiles):
                    c0 = t * P
                    ps = g_ps.tile([P, U], F32)
                    nc.tensor.matmul(
                        ps[:, :], xt0[:, c0 : c0 + P], wkt[:, 0:U],
                        start=True, stop=False,
                    )
                    nc.tensor.matmul(
                        ps[:, :], xt1[:, c0 : c0 + P], wkt[:, U : 2 * U],
                        start=False, stop=True,
                    )
                    nc.scalar.copy(hb[:, t * U : (t + 1) * U], ps[:, :])
                # pair layout: node n -> pair row n>>1, half n&1.
                # nodes n0..n0+ncnt-1 -> pair rows n0/2 .. n0/2+ncnt/2-1
                dr = h_pair[n0 // 2 : (n0 + ncnt) // 2, :].rearrange(
                    "(o r) su -> (r su) o", r=P // 2
                ).rearrange("(r s u) o -> (r s) o u", s=2, u=U)
                srcv = hb[:].rearrange("p (o u) -> p o u", u=U)[:, 0:ntiles, :]
                h_writes.append(nc.sync.dma_start(dr, srcv))

            # ---------------- phase 2: gather + median ----------------
            ctx1.close()
            g_net = ctx.enter_context(tc.tile_pool(name="g_net", bufs=net_bufs))
            g_srt = ctx.enter_context(tc.tile_pool(name="g_srt", bufs=2))
            g_idx = ctx.enter_context(tc.tile_pool(name="g_idx", bufs=2))
            g_out = ctx.enter_context(tc.tile_pool(name="g_out", bufs=2))

            nc.gpsimd.load_library(library_config.mlp)
            n_g = 0
            for c in range(NCHUNK):
                ia = g_idx.tile([P, IDXCOLS], I16, tag="ia")
                pa = g_idx.tile([P, J], U8, tag="pa")
                nc.sync.dma_start(ia[:], idx[c])
                nc.sync.dma_start(pa[:], par[c])
                pt = g_net.tile([P, J * 2 * U], F16, tag="pt")
                # one gather call per split: 256B pair rows, k-major cols
                per = NIDX // gather_split
                assert per % P == 0
                for gsp in range(gather_split):
                    jj0 = gsp * (per // P)
                    jj1 = (gsp + 1) * (per // P)
                    g = nc.gpsimd.dma_gather(
                        pt[:, jj0 * 2 * U : jj1 * 2 * U].rearrange(
                            "p (j e) -> p j e", e=2 * U
                        ),
                        h_pair[:],
                        ia[:, gsp * per // 16 : (gsp + 1) * per // 16],
                        per,
                        per,
                        2 * U,
                        single_packet=False,
                    )
                    if n_g == 0:
                        for w in h_writes:
                            add_dep_helper(
                                g.ins, w.ins,
                                reason="gather waits for h DRAM writes",
                            )
                    n_g += 1
                # resolve pair parity in place: even half (s=0) is the
                # message home; overwrite with odd half where parity=1
                p4 = pt[:].rearrange("p (j s u) -> p j s u", s=2, u=U)
                mask = pa[:].unsqueeze(2).broadcast_to([P, J, U])
                nc.vector.copy_predicated(
                    out=p4[:, :, 0, :], mask=mask, data=p4[:, :, 1, :]
                )

                ra = g_srt.tile([P, K * BU], F16, tag="ra")
                rb = g_srt.tile([P, K * BU], F16, tag="rb")

                # stage 1 of the Batcher network reads the strided message
                # view (s=0 halves of pt) and writes compact k-major planes
                msg = pt[:].rearrange(
                    "p (hi r b s u) -> p hi r b s u", hi=16, r=2, b=B, s=2, u=U
                )[:, :, :, :, 0, :]
                vd = ra[:].rearrange("p (hi r bu) -> p hi r bu", r=2, bu=BU)
                lo_s = msg[:, :, 0, :, :]
                hi_s = msg[:, :, 1, :, :]
                nc.vector.tensor_tensor(
                    out=vd[:, :, 0, :].rearrange("p hi (b u) -> p hi b u", u=U),
                    in0=lo_s, in1=hi_s, op=mybir.AluOpType.min,
                )
                nc.vector.tensor_tensor(
                    out=vd[:, :, 1, :].rearrange("p hi (b u) -> p hi b u", u=U),
                    in0=lo_s, in1=hi_s, op=mybir.AluOpType.max,
                )

                # Batcher network stages 2..10 over both halves, ping-pong
                src, dst = ra, rb
                for sp in SORT16_STAGES[1:]:
                    f = sp["f"]
                    ni = 16 // f
                    i_full = sp["i"] == (0, ni, 1)
                    d = sp["d"]
                    di, dr = d // f, d % f
                    r_vals = list(range(*sp["r"]))
                    if r_vals[-1] + dr >= f:
                        assert all(rv + dr >= f for rv in r_vals), sp
                        di, dr = di + 1, dr - f
                    r_sl = slice(*sp["r"])
                    hi_r = slice(sp["r"][0] + dr, sp["r"][1] + dr, sp["r"][2])
                    if i_full and di == 0:
                        vs = src[:].rearrange("p (hi r bu) -> p hi r bu", r=f, bu=BU)
                        vd = dst[:].rearrange("p (hi r bu) -> p hi r bu", r=f, bu=BU)
                        lo_s = vs[:, :, r_sl, :]
                        hi_s = vs[:, :, hi_r, :]
                        nc.vector.tensor_tensor(
                            out=vd[:, :, r_sl, :], in0=lo_s, in1=hi_s,
                            op=mybir.AluOpType.min,
                        )
                        nc.vector.tensor_tensor(
                            out=vd[:, :, hi_r, :], in0=lo_s, in1=hi_s,
                            op=mybir.AluOpType.max,
                        )
                    else:
                        i_sl = slice(*sp["i"])
                        hi_i = slice(sp["i"][0] + di, sp["i"][1] + di, sp["i"][2])
                        vs = src[:].rearrange(
                            "p (hh i r bu) -> p hh i r bu", hh=2, i=ni, r=f, bu=BU
                        )
                        vd = dst[:].rearrange(
                            "p (hh i r bu) -> p hh i r bu", hh=2, i=ni, r=f, bu=BU
                        )
                        lo_s = vs[:, :, i_sl, r_sl, :]
                        hi_s = vs[:, :, hi_i, hi_r, :]
                        nc.vector.tensor_tensor(
                            out=vd[:, :, i_sl, r_sl, :], in0=lo_s, in1=hi_s,
                            op=mybir.AluOpType.min,
                        )
                        nc.vector.tensor_tensor(
                            out=vd[:, :, hi_i, hi_r, :], in0=lo_s, in1=hi_s,
                            op=mybir.AluOpType.max,
                        )
                    vks = src[:].rearrange("p (hh kk bu) -> p hh kk bu", hh=2, kk=16)
                    vkd = dst[:].rearrange("p (hh kk bu) -> p hh kk bu", hh=2, kk=16)
                    for cpsl in sp["cp"]:
                        ks = slice(*cpsl)
                        nc.scalar.copy(vkd[:, :, ks, :], vks[:, :, ks, :])
                    src, dst = dst, src

                # anti-diagonal merge of the two sorted 16-plane halves
                vk = src[:].rearrange("p (k bu) -> p k bu", k=K)
                vo = dst[:].rearrange("p (k bu) -> p k bu", k=K)
                A = vk[:, 0:16, :]
                Brev = vk[:, 31:15:-1, :]
                nc.vector.tensor_tensor(
                    out=vo[:, 0:16, :], in0=A, in1=Brev, op=mybir.AluOpType.max
                )
                nc.vector.tensor_tensor(
                    out=vk[:, 0:16, :], in0=A, in1=Brev, op=mybir.AluOpType.min
                )
                # min/max trees over the 16 planes: 16 -> 8 -> 4 -> 2 -> 1.
                # low ends in vk plane 0, up in vo plane 0.
                for buf, op in ((vk, mybir.AluOpType.max), (vo, mybir.AluOpType.min)):
                    w = 16
                    while w > 1:
                        h = w // 2
                        nc.vector.tensor_tensor(
                            out=buf[:, 0:h, :], in0=buf[:, 0:h, :],
                            in1=buf[:, h:w, :], op=op,
                        )
                        w = h
                med = g_out.tile([P, BU], F32, tag="med")
                nc.gpsimd.tensor_tensor(
                    out=med[:], in0=vk[:, 0, :], in1=vo[:, 0, :],
                    op=mybir.AluOpType.add,
                )
                nc.sync.dma_start(out[c], med[:])

    nc.compile()
    return nc


def _prep_inputs(x, neighbors, kern, num_cores=NUM_CORES, C=CHUNK):
    nrows = x.shape[0]
    total = neighbors.shape[0]
    shard = (total + num_cores - 1) // num_cores
    NCHUNK = (shard + C - 1) // C
    shard_pad = NCHUNK * C
    B = C // P
    NIDX = C * K
    IDXCOLS = NIDX // 16
    J = NIDX // P

    nrows_pad = ((nrows + P - 1) // P) * P
    xT = np.zeros((FEAT, nrows_pad), dtype=np.float16)
    xT[:, :nrows] = x.T
    # fold the midpoint *0.5 into the weights (median is scale-equivariant)
    wk = np.ascontiguousarray(kern * 0.5).astype(np.float16)

    in_maps = []
    for core in range(num_cores):
        n0 = core * shard
        nbr = np.zeros((shard_pad, K), dtype=np.int64)
        real = min(shard, total - n0)
        nbr[:real] = neighbors[n0 : n0 + real]
        # i = ((k*B + b)*128 + p) enumerates (plane k, block b, partition p)
        nb = (
            nbr.reshape(NCHUNK, B, P, K).transpose(0, 3, 1, 2).reshape(NCHUNK, NIDX)
        )
        pairs = (nb >> 1).astype(np.int16)
        parity = (nb & 1).astype(np.uint8)
        # logical index i lives at [i%16, i//16]; replicated to all eight
        # 16-partition groups (Q7 core pairs read their own)
        idxarr = np.tile(
            pairs.reshape(NCHUNK, IDXCOLS, 16).transpose(0, 2, 1), (1, P // 16, 1)
        )
        pararr = np.ascontiguousarray(
            parity.reshape(NCHUNK, J, P).transpose(0, 2, 1)
        )
        in_maps.append({"xT": xT, "wk": wk, "idx": idxarr, "par": pararr})
    meta = dict(shard=shard, shard_pad=shard_pad, NCHUNK=NCHUNK, C=C, total=total)
    return in_maps, meta


def _unshard_output(results, meta, num_cores=NUM_CORES):
    outs = []
    for core in range(num_cores):
        o = results[core]["out"]  # [NCHUNK, P, B*U]
        NCHUNK, _, BU = o.shape
        B = BU // U
        o = (
            o.reshape(NCHUNK, P, B, U)
            .transpose(0, 2, 1, 3)
            .reshape(meta["shard_pad"], U)
        )
        outs.append(o[: meta["shard"]])
    return np.concatenate(outs, axis=0)[: meta["total"]]


_CACHE = {}


def kernel(x, neighbors, kernel):
    """Full inputs in, full output out. Shards nodes across 8 NeuronCores."""
    x = np.asarray(x, dtype=np.float32)
    neighbors_np = np.asarray(neighbors)
    kern = np.asarray(kernel, dtype=np.float32)
    assert x.shape[1] == FEAT and kern.shape == (FEAT, U)
    assert neighbors_np.shape[1] == K

    in_maps, meta = _prep_inputs(x, neighbors_np, kern)
    key = (x.shape[0], meta["shard_pad"], meta["C"])
    if key not in _CACHE:
        _CACHE[key] = build_kernel(x.shape[0], meta["shard_pad"], meta["C"])
    nc = _CACHE[key]
    res = bass_utils.run_bass_kernel_spmd(
        nc, in_maps, core_ids=list(range(NUM_CORES))
    )
    return _unshard_output(res.results, meta)


# revision 19
# speedup vs baseline: 1.8208x; 1.0173x over previous
"""MedianConvolution (gnn message passing) — Trainium2 Bass kernel, 8 cores.

Computes: h = x @ kernel; msg = h[neighbors]; out = exact midpoint median
over the K=32 neighbor axis (ranks 15,16 of the sort), i.e.
tfp percentile(q=50, interpolation='midpoint').

Distribution: nodes (rows of neighbors) are sharded across the 8 NeuronCores;
every core computes the full h = x @ kernel on-device (x/kernel replicated,
fp16 inputs, fp32 PSUM accumulate) and gathers only its own node shard's
neighbor rows.

Key layout trick: h is stored in DRAM as PAIRS h_pair[r] = [h[2r] | h[2r+1]]
(fp16, 256B rows). dma_gather requires 256B-aligned elements and int16
indices; the pair layout satisfies both (idx = node>>1 <= 24999) with ONE
descriptor per neighbor instead of the two (lo/hi halves) the fp32 baseline
needed. The wrong pair half is discarded on-chip with a single
copy_predicated keyed on a host-uploaded parity mask.

The whole median datapath runs in fp16: TensorTensor min/max supports the
DVE 2x_1p fast mode (2-byte dtypes) for 2x throughput, and the final
midpoint is exact up to fp16 rounding (~0.05%), far inside the 2e-2 gate.
The x0.5 of the midpoint is folded into the GEMM weights (median is
scale-equivariant for positive scales).

Per-core SPMD program:
  phase 1  GEMM: xT fp16 [256, N] x wk fp16 [256, 64] -> PSUM fp32, copied
           to fp16 and DMAed into the pair layout.
  phase 2  per chunk of C=256 shard nodes: one 8192-index dma_gather pulls
           the 256B pair rows for all 32 neighbor planes (k-major), a
           copy_predicated resolves pair parity in place, a Batcher
           odd-even mergesort sorts planes 0-15 and 16-31 (fp16 TT min/max,
           untouched planes copied on the Scalar engine), the 32-way median
           pair comes from the anti-diagonal identity
              low = max_i min(A_i, B_15-i),  up = min_i max(A_i, B_15-i)
           via two TT ops + two min/max trees, and low+up (already scaled
           by 0.5) is written out in fp32.
"""
from contextlib import ExitStack

import numpy as np

import concourse.bass as bass
import concourse.tile as tile
from concourse import bacc, bass_utils, library_config, mybir
from concourse.tile_rust import add_dep_helper

F32 = mybir.dt.float32
F16 = mybir.dt.float16
I16 = mybir.dt.int16
U8 = mybir.dt.uint8
P = 128
U = 64  # units
K = 32  # neighbors
FEAT = 256
N_NODES = 50000
NUM_CORES = 8
CHUNK = 256  # shard nodes per chunk
NET_BUFS = 4
GATHER_SPLIT = 1  # dma_gather calls per chunk

# Batcher odd-even mergesort(16) stages; verified against np.sort via the
# 0-1 principle. Each stage: comparators (k, k+d) for k = i*f + r over the
# slices below, applied to both 16-plane halves. cp = untouched plane
# slices (copied to the ping-pong destination).
SORT16_STAGES = [
    dict(f=2, i=(0, 8, 1), r=(0, 1, 1), d=1, cp=[]),
    dict(f=4, i=(0, 4, 1), r=(0, 2, 1), d=2, cp=[]),
    dict(f=4, i=(0, 4, 1), r=(1, 2, 1), d=1, cp=[(0, 16, 4), (3, 16, 4)]),
    dict(f=8, i=(0, 2, 1), r=(0, 4, 1), d=4, cp=[]),
    dict(f=8, i=(0, 2, 1), r=(2, 4, 1), d=2,
         cp=[(0, 16, 8), (1, 16, 8), (6, 16, 8), (7, 16, 8)]),
    dict(f=8, i=(0, 2, 1), r=(1, 6, 2), d=1, cp=[(0, 16, 8), (7, 16, 8)]),
    dict(f=16, i=(0, 1, 1), r=(0, 8, 1), d=8, cp=[]),
    dict(f=16, i=(0, 1, 1), r=(4, 8, 1), d=4, cp=[(0, 4, 1), (12, 16, 1)]),
    dict(f=4, i=(0, 3, 1), r=(2, 4, 1), d=2, cp=[(0, 2, 1), (14, 16, 1)]),
    dict(f=2, i=(0, 7, 1), r=(1, 2, 1), d=1, cp=[(0, 16, 15)]),
]


def build_kernel(nrows, shard_nodes, C, num_cores=NUM_CORES, gemm_super=2048,
                 net_bufs=NET_BUFS, gather_split=GATHER_SPLIT):
    nrows = ((nrows + P - 1) // P) * P  # host pads xT to the same size
    NPAIR = nrows // 2
    NCHUNK = shard_nodes // C
    assert NCHUNK * C == shard_nodes
    B = C // P
    BU = B * U
    NIDX = C * K          # gather indices (pair rows) per chunk
    IDXCOLS = NIDX // 16
    J = NIDX // P         # gathered columns per chunk (= K * B)

    nc = bacc.Bacc(
        "TRN2",
        target_bir_lowering=False,
        debug=False,
        num_devices=num_cores,
    )

    xT = nc.dram_tensor("xT", [FEAT, nrows], F16, kind="ExternalInput").ap()
    wk = nc.dram_tensor("wk", [FEAT, U], F16, kind="ExternalInput").ap()
    idx = nc.dram_tensor("idx", [NCHUNK, P, IDXCOLS], I16, kind="ExternalInput").ap()
    par = nc.dram_tensor("par", [NCHUNK, P, J], U8, kind="ExternalInput").ap()
    out = nc.dram_tensor("out", [NCHUNK, P, BU], F32, kind="ExternalOutput").ap()
    h_pair = nc.dram_tensor("h_pair", [NPAIR, 2 * U], F16, kind="Internal").ap()

    with tile.TileContext(nc) as tc:
        with ExitStack() as ctx:
            # ---------------- phase 1: GEMM ----------------
            ctx1 = ctx.enter_context(ExitStack())
            g_x = ctx1.enter_context(tc.tile_pool(name="g_x", bufs=3))
            g_w = ctx1.enter_context(tc.tile_pool(name="g_w", bufs=1))
            g_h = ctx1.enter_context(tc.tile_pool(name="g_h", bufs=3))
            g_ps = ctx1.enter_context(tc.tile_pool(name="g_ps", bufs=8, space="PSUM"))

            wkt = g_w.tile([P, 2 * U], F16)
            nc.sync.dma_start(wkt[:, 0:U], wk[0:P, :])
            nc.sync.dma_start(wkt[:, U : 2 * U], wk[P : 2 * P, :])

            h_writes = []
            S = gemm_super
            n_super = (nrows + S - 1) // S
            for s in range(n_super):
                n0 = s * S
                ncnt = min(S, nrows - n0)
                assert ncnt % P == 0
                ntiles = ncnt // P
                xt0 = g_x.tile([P, S], F16, tag="xt0")
                xt1 = g_x.tile([P, S], F16, tag="xt1")
                nc.sync.dma_start(xt0[:, 0:ncnt], xT[0:P, n0 : n0 + ncnt])
                nc.sync.dma_start(xt1[:, 0:ncnt], xT[P : 2 * P, n0 : n0 + ncnt])
                hb = g_h.tile([P, (S // P) * U], F16, tag="hb")
                for t in range(ntiles):
                    c0 = t * P
                    ps = g_ps.tile([P, U], F32)
                    nc.tensor.matmul(
                        ps[:, :], xt0[:, c0 : c0 + P], wkt[:, 0:U],
                        start=True, stop=False,
                    )
                    nc.tensor.matmul(
                        ps[:, :], xt1[:, c0 : c0 + P], wkt[:, U : 2 * U],
                        start=False, stop=True,
                    )
                    nc.scalar.copy(hb[:, t * U : (t + 1) * U], ps[:, :])
                # pair layout: node n -> pair row n>>1, half n&1.
                # nodes n0..n0+ncnt-1 -> pair rows n0/2 .. n0/2+ncnt/2-1
                dr = h_pair[n0 // 2 : (n0 + ncnt) // 2, :].rearrange(
                    "(o r) su -> (r su) o", r=P // 2
                ).rearrange("(r s u) o -> (r s) o u", s=2, u=U)
                srcv = hb[:].rearrange("p (o u) -> p o u", u=U)[:, 0:ntiles, :]
                h_writes.append(nc.sync.dma_start(dr, srcv))

            # ---------------- phase 2: gather + median ----------------
            ctx1.close()
            g_net = ctx.enter_context(tc.tile_pool(name="g_net", bufs=net_bufs))
            g_srt = ctx.enter_context(tc.tile_pool(name="g_srt", bufs=4))
            g_idx = ctx.enter_context(tc.tile_pool(name="g_idx", bufs=2))
            g_out = ctx.enter_context(tc.tile_pool(name="g_out", bufs=2))

            nc.gpsimd.load_library(library_config.mlp)
            n_g = 0
            for c in range(NCHUNK):
                ia = g_idx.tile([P, IDXCOLS], I16, tag="ia")
                pa = g_idx.tile([P, J], U8, tag="pa")
                nc.sync.dma_start(ia[:], idx[c])
                nc.sync.dma_start(pa[:], par[c])
                pt = g_net.tile([P, J * 2 * U], F16, tag="pt")
                # one gather call per split: 256B pair rows, k-major cols.
                # chunk 0 is split finer so parity resolution starts before
                # the whole first gather lands (shortens the phase-1 ramp)
                split = 4 if c == 0 else gather_split
                per = NIDX // split
                assert per % P == 0
                p4 = pt[:].rearrange("p (j s u) -> p j s u", s=2, u=U)
                for gsp in range(split):
                    jj0 = gsp * (per // P)
                    jj1 = (gsp + 1) * (per // P)
                    g = nc.gpsimd.dma_gather(
                        pt[:, jj0 * 2 * U : jj1 * 2 * U].rearrange(
                            "p (j e) -> p j e", e=2 * U
                        ),
                        h_pair[:],
                        ia[:, gsp * per // 16 : (gsp + 1) * per // 16],
                        per,
                        per,
                        2 * U,
                        single_packet=False,
                    )
                    if n_g == 0:
                        for w in h_writes:
                            add_dep_helper(
                                g.ins, w.ins,
                                reason="gather waits for h DRAM writes",
                            )
                    n_g += 1
                    # resolve pair parity in place: even half (s=0) is the
                    # message home; overwrite with odd half where parity=1
                    mask = pa[:, jj0:jj1].unsqueeze(2).broadcast_to(
                        [P, jj1 - jj0, U]
                    )
                    nc.vector.copy_predicated(
                        out=p4[:, jj0:jj1, 0, :], mask=mask,
                        data=p4[:, jj0:jj1, 1, :],
                    )

                ra = g_srt.tile([P, K * BU], F16, tag="ra")
                rb = g_srt.tile([P, K * BU], F16, tag="rb")

                # stage 1 of the Batcher network reads the strided message
                # view (s=0 halves of pt) and writes compact k-major planes
                msg = pt[:].rearrange(
                    "p (hi r b s u) -> p hi r b s u", hi=16, r=2, b=B, s=2, u=U
                )[:, :, :, :, 0, :]
                vd = ra[:].rearrange("p (hi r bu) -> p hi r bu", r=2, bu=BU)
                lo_s = msg[:, :, 0, :, :]
                hi_s = msg[:, :, 1, :, :]
                nc.vector.tensor_tensor(
                    out=vd[:, :, 0, :].rearrange("p hi (b u) -> p hi b u", u=U),
                    in0=lo_s, in1=hi_s, op=mybir.AluOpType.min,
                )
                nc.vector.tensor_tensor(
                    out=vd[:, :, 1, :].rearrange("p hi (b u) -> p hi b u", u=U),
                    in0=lo_s, in1=hi_s, op=mybir.AluOpType.max,
                )

                # Batcher network stages 2..10 over both halves, ping-pong
                src, dst = ra, rb
                for sp in SORT16_STAGES[1:]:
                    veng = nc.vector
                    f = sp["f"]
                    ni = 16 // f
                    i_full = sp["i"] == (0, ni, 1)
                    d = sp["d"]
                    di, dr = d // f, d % f
                    r_vals = list(range(*sp["r"]))
                    if r_vals[-1] + dr >= f:
                        assert all(rv + dr >= f for rv in r_vals), sp
                        di, dr = di + 1, dr - f
                    r_sl = slice(*sp["r"])
                    hi_r = slice(sp["r"][0] + dr, sp["r"][1] + dr, sp["r"][2])
                    if i_full and di == 0:
                        vs = src[:].rearrange("p (hi r bu) -> p hi r bu", r=f, bu=BU)
                        vd = dst[:].rearrange("p (hi r bu) -> p hi r bu", r=f, bu=BU)
                        lo_s = vs[:, :, r_sl, :]
                        hi_s = vs[:, :, hi_r, :]
                        veng.tensor_tensor(
                            out=vd[:, :, r_sl, :], in0=lo_s, in1=hi_s,
                            op=mybir.AluOpType.min,
                        )
                        veng.tensor_tensor(
                            out=vd[:, :, hi_r, :], in0=lo_s, in1=hi_s,
                            op=mybir.AluOpType.max,
                        )
                    else:
                        i_sl = slice(*sp["i"])
                        hi_i = slice(sp["i"][0] + di, sp["i"][1] + di, sp["i"][2])
                        vs = src[:].rearrange(
                            "p (hh i r bu) -> p hh i r bu", hh=2, i=ni, r=f, bu=BU
                        )
                        vd = dst[:].rearrange(
                            "p (hh i r bu) -> p hh i r bu", hh=2, i=ni, r=f, bu=BU
                        )
                        lo_s = vs[:, :, i_sl, r_sl, :]
                        hi_s = vs[:, :, hi_i, hi_r, :]
                        veng.tensor_tensor(
                            out=vd[:, :, i_sl, r_sl, :], in0=lo_s, in1=hi_s,
                            op=mybir.AluOpType.min,
                        )
                        veng.tensor_tensor(
                            out=vd[:, :, hi_i, hi_r, :], in0=lo_s, in1=hi_s,
                            op=mybir.AluOpType.max,
                        )
                    vks = src[:].rearrange("p (hh kk bu) -> p hh kk bu", hh=2, kk=16)
                    vkd = dst[:].rearrange("p (hh kk bu) -> p hh kk bu", hh=2, kk=16)
                    for cpsl in sp["cp"]:
                        ks = slice(*cpsl)
                        nc.scalar.copy(vkd[:, :, ks, :], vks[:, :, ks, :])
                    src, dst = dst, src

                # anti-diagonal merge of the two sorted 16-plane halves
                # (min/max must run on DVE: the Pool ucode only implements
                # add/mult). Results go to separate small tiles so ra/rb
                # free up for the next chunk once the antidiag ops retire.
                eng = nc.vector
                vk = src[:].rearrange("p (k bu) -> p k bu", k=K)
                A = vk[:, 0:16, :]
                Brev = vk[:, 31:15:-1, :]
                tlo = g_out.tile([P, 16 * BU], F16, tag="tlo")
                tup = g_out.tile([P, 16 * BU], F16, tag="tup")
                vlo = tlo[:].rearrange("p (k bu) -> p k bu", k=16)
                vup = tup[:].rearrange("p (k bu) -> p k bu", k=16)
                eng.tensor_tensor(
                    out=vup[:, :, :], in0=A, in1=Brev, op=mybir.AluOpType.max
                )
                eng.tensor_tensor(
                    out=vlo[:, :, :], in0=A, in1=Brev, op=mybir.AluOpType.min
                )
                # min/max trees over the 16 planes: 16 -> 8 -> 4 -> 2 -> 1.
                # low ends in vlo plane 0, up in vup plane 0.
                for buf, op in ((vlo, mybir.AluOpType.max), (vup, mybir.AluOpType.min)):
                    w = 16
                    while w > 1:
                        h = w // 2
                        eng.tensor_tensor(
                            out=buf[:, 0:h, :], in0=buf[:, 0:h, :],
                            in1=buf[:, h:w, :], op=op,
                        )
                        w = h
                med = g_out.tile([P, BU], F32, tag="med")
                nc.gpsimd.tensor_tensor(
                    out=med[:], in0=vlo[:, 0, :], in1=vup[:, 0, :],
                    op=mybir.AluOpType.add,
                )
                nc.sync.dma_start(out[c], med[:])

    nc.compile()
    return nc


def _prep_inputs(x, neighbors, kern, num_cores=NUM_CORES, C=CHUNK):
    nrows = x.shape[0]
    total = neighbors.shape[0]
    shard = (total + num_cores - 1) // num_cores
    NCHUNK = (shard + C - 1) // C
    shard_pad = NCHUNK * C
    B = C // P
    NIDX = C * K
    IDXCOLS = NIDX // 16
    J = NIDX // P

    nrows_pad = ((nrows + P - 1) // P) * P
    xT = np.zeros((FEAT, nrows_pad), dtype=np.float16)
    xT[:, :nrows] = x.T
    # fold the midpoint *0.5 into the weights (median is scale-equivariant)
    wk = np.ascontiguousarray(kern * 0.5).astype(np.float16)

    in_maps = []
    for core in range(num_cores):
        n0 = core * shard
        nbr = np.zeros((shard_pad, K), dtype=np.int64)
        real = min(shard, total - n0)
        nbr[:real] = neighbors[n0 : n0 + real]
        # i = ((k*B + b)*128 + p) enumerates (plane k, block b, partition p)
        nb = (
            nbr.reshape(NCHUNK, B, P, K).transpose(0, 3, 1, 2).reshape(NCHUNK, NIDX)
        )
        pairs = (nb >> 1).astype(np.int16)
        parity = (nb & 1).astype(np.uint8)
        # logical index i lives at [i%16, i//16]; replicated to all eight
        # 16-partition groups (Q7 core pairs read their own)
        idxarr = np.tile(
            pairs.reshape(NCHUNK, IDXCOLS, 16).transpose(0, 2, 1), (1, P // 16, 1)
        )
        pararr = np.ascontiguousarray(
            parity.reshape(NCHUNK, J, P).transpose(0, 2, 1)
        )
        in_maps.append({"xT": xT, "wk": wk, "idx": idxarr, "par": pararr})
    meta = dict(shard=shard, shard_pad=shard_pad, NCHUNK=NCHUNK, C=C, total=total)
    return in_maps, meta


def _unshard_output(results, meta, num_cores=NUM_CORES):
    outs = []
    for core in range(num_cores):
        o = results[core]["out"]  # [NCHUNK, P, B*U]
        NCHUNK, _, BU = o.shape
        B = BU // U
        o = (
            o.reshape(NCHUNK, P, B, U)
            .transpose(0, 2, 1, 3)
            .reshape(meta["shard_pad"], U)
        )
        outs.append(o[: meta["shard"]])
    return np.concatenate(outs, axis=0)[: meta["total"]]


_CACHE = {}


def kernel(x, neighbors, kernel):
    """Full inputs in, full output out. Shards nodes across 8 NeuronCores."""
    x = np.asarray(x, dtype=np.float32)
    neighbors_np = np.asarray(neighbors)
    kern = np.asarray(kernel, dtype=np.float32)
    assert x.shape[1] == FEAT and kern.shape == (FEAT, U)
    assert neighbors_np.shape[1] == K

    in_maps, meta = _prep_inputs(x, neighbors_np, kern)
    key = (x.shape[0], meta["shard_pad"], meta["C"])
    if key not in _CACHE:
        _CACHE[key] = build_kernel(x.shape[0], meta["shard_pad"], meta["C"])
    nc = _CACHE[key]
    res = bass_utils.run_bass_kernel_spmd(
        nc, in_maps, core_ids=list(range(NUM_CORES))
    )
    return _unshard_output(res.results, meta)


# revision 24
# speedup vs baseline: 1.8762x; 1.0304x over previous
"""MedianConvolution (gnn message passing) — Trainium2 Bass kernel, 8 cores.

Computes: h = x @ kernel; msg = h[neighbors]; out = exact midpoint median
over the K=32 neighbor axis (ranks 15,16 of the sort), i.e.
tfp percentile(q=50, interpolation='midpoint').

Distribution: nodes (rows of neighbors) are sharded across the 8 NeuronCores;
every core computes the full h = x @ kernel on-device (x/kernel replicated,
fp16 inputs, fp32 PSUM accumulate) and gathers only its own node shard's
neighbor rows.

Key layout trick: h is stored in DRAM as PAIRS h_pair[r] = [h[2r] | h[2r+1]]
(fp16, 256B rows). dma_gather requires 256B-aligned elements and int16
indices; the pair layout satisfies both (idx = node>>1 <= 24999) with ONE
descriptor per neighbor instead of the two (lo/hi halves) an fp32 layout
needs. The wrong pair half is discarded on-chip with a single
copy_predicated keyed on a host-uploaded parity mask.

The whole median datapath runs in fp16: TensorTensor min/max supports the
DVE 2x_1p fast mode (2-byte dtypes) for 2x throughput, and the final
midpoint is exact up to fp16 rounding (~0.05%), far inside the 2e-2 gate.
The x0.5 of the midpoint is folded into the GEMM weights (median is
scale-equivariant for positive scales). min/max only exist on the DVE
(the GPSIMD ucode implements only add/mult), so the whole median pipeline
lives there; plane copies go to the Scalar engine, descriptor generation
to GPSIMD, and the node shard is processed in 512-node chunks (one 128-node
tail chunk) to amortize per-instruction overheads on the bottleneck DVE.

Per-core SPMD program:
  phase 1  GEMM: xT fp16 [256, N] x wk fp16 [256, 64] -> PSUM fp32, copied
           to fp16 and DMAed into the pair layout.
  phase 2  per chunk of C shard nodes: dma_gather pulls the 256B pair rows
           for all 32 neighbor planes (k-major, <=8192 indices per call), a
           copy_predicated resolves pair parity in place, a Batcher
           odd-even mergesort sorts planes 0-15 and 16-31 (fp16 TT min/max,
           untouched planes copied on the Scalar engine), the 32-way median
           pair comes from the anti-diagonal identity
              low = max_i min(A_i, B_15-i),  up = min_i max(A_i, B_15-i)
           via two TT ops + two min/max trees, and low+up (already scaled
           by 0.5) is written out in fp32.
"""
from contextlib import ExitStack

import numpy as np

import concourse.bass as bass
import concourse.tile as tile
from concourse import bacc, bass_utils, library_config, mybir
from concourse.tile_rust import add_dep_helper

F32 = mybir.dt.float32
F16 = mybir.dt.float16
I16 = mybir.dt.int16
U8 = mybir.dt.uint8
P = 128
U = 64  # units
K = 32  # neighbors
FEAT = 256
N_NODES = 50000
NUM_CORES = 8
MEGA = 512        # main chunk size (nodes); tail chunk is the padded rest
NET_BUFS = 2
MAX_GATHER = 8192  # max indices per dma_gather call (HW-validated)

# Batcher odd-even mergesort(16) stages; verified against np.sort via the
# 0-1 principle. Each stage: comparators (k, k+d) for k = i*f + r over the
# slices below, applied to both 16-plane halves. cp = untouched plane
# slices (copied to the ping-pong destination).
SORT16_STAGES = [
    dict(f=2, i=(0, 8, 1), r=(0, 1, 1), d=1, cp=[]),
    dict(f=4, i=(0, 4, 1), r=(0, 2, 1), d=2, cp=[]),
    dict(f=4, i=(0, 4, 1), r=(1, 2, 1), d=1, cp=[(0, 16, 4), (3, 16, 4)]),
    dict(f=8, i=(0, 2, 1), r=(0, 4, 1), d=4, cp=[]),
    dict(f=8, i=(0, 2, 1), r=(2, 4, 1), d=2,
         cp=[(0, 16, 8), (1, 16, 8), (6, 16, 8), (7, 16, 8)]),
    dict(f=8, i=(0, 2, 1), r=(1, 6, 2), d=1, cp=[(0, 16, 8), (7, 16, 8)]),
    dict(f=16, i=(0, 1, 1), r=(0, 8, 1), d=8, cp=[]),
    dict(f=16, i=(0, 1, 1), r=(4, 8, 1), d=4, cp=[(0, 4, 1), (12, 16, 1)]),
    dict(f=4, i=(0, 3, 1), r=(2, 4, 1), d=2, cp=[(0, 2, 1), (14, 16, 1)]),
    dict(f=2, i=(0, 7, 1), r=(1, 2, 1), d=1, cp=[(0, 16, 15)]),
]


def _chunk_grid(shard):
    """Chunk sizes covering `shard` nodes: a small first chunk (so the
    gather->sort pipeline fills quickly after the GEMM), MEGA-chunks for
    the bulk, and a padded tail."""
    grid = []
    rest = shard
    if rest > 2 * P + MEGA:
        grid.append(2 * P)
        rest -= 2 * P
    grid += [MEGA] * (rest // MEGA)
    rem = rest - (rest // MEGA) * MEGA
    if rem:
        grid.append(((rem + P - 1) // P) * P)
    return grid


def build_kernel(nrows, shard, num_cores=NUM_CORES, gemm_super=2048,
                 net_bufs=NET_BUFS):
    nrows = ((nrows + P - 1) // P) * P  # host pads xT to the same size
    NPAIR = nrows // 2
    grid = _chunk_grid(shard)
    IDXTOT = sum(Cc * K // 16 for Cc in grid)
    JTOT = sum(Cc * K // P for Cc in grid)
    BUTOT = sum(Cc // 2 for Cc in grid)

    nc = bacc.Bacc(
        "TRN2",
        target_bir_lowering=False,
        debug=False,
        num_devices=num_cores,
    )

    xT = nc.dram_tensor("xT", [FEAT, nrows], F16, kind="ExternalInput").ap()
    wk = nc.dram_tensor("wk", [FEAT, U], F16, kind="ExternalInput").ap()
    idx = nc.dram_tensor("idx", [P, IDXTOT], I16, kind="ExternalInput").ap()
    par = nc.dram_tensor("par", [P, JTOT], U8, kind="ExternalInput").ap()
    out = nc.dram_tensor("out", [P, BUTOT], F32, kind="ExternalOutput").ap()
    h_pair = nc.dram_tensor("h_pair", [NPAIR, 2 * U], F16, kind="Internal").ap()

    with tile.TileContext(nc) as tc:
        with ExitStack() as ctx:
            # ---------------- phase 1: GEMM ----------------
            ctx1 = ctx.enter_context(ExitStack())
            g_x = ctx1.enter_context(tc.tile_pool(name="g_x", bufs=3))
            g_w = ctx1.enter_context(tc.tile_pool(name="g_w", bufs=1))
            g_h = ctx1.enter_context(tc.tile_pool(name="g_h", bufs=3))
            g_ps = ctx1.enter_context(tc.tile_pool(name="g_ps", bufs=8, space="PSUM"))

            wkt = g_w.tile([P, 2 * U], F16)
            nc.sync.dma_start(wkt[:, 0:U], wk[0:P, :])
            nc.sync.dma_start(wkt[:, U : 2 * U], wk[P : 2 * P, :])

            h_writes = []
            S = gemm_super
            n_super = (nrows + S - 1) // S
            for s in range(n_super):
                n0 = s * S
                ncnt = min(S, nrows - n0)
                assert ncnt % P == 0
                ntiles = ncnt // P
                xt0 = g_x.tile([P, S], F16, tag="xt0")
                xt1 = g_x.tile([P, S], F16, tag="xt1")
                nc.sync.dma_start(xt0[:, 0:ncnt], xT[0:P, n0 : n0 + ncnt])
                nc.sync.dma_start(xt1[:, 0:ncnt], xT[P : 2 * P, n0 : n0 + ncnt])
                hb = g_h.tile([P, (S // P) * U], F16, tag="hb")
                for t in range(ntiles):
                    c0 = t * P
                    ps = g_ps.tile([P, U], F32)
                    nc.tensor.matmul(
                        ps[:, :], xt0[:, c0 : c0 + P], wkt[:, 0:U],
                        start=True, stop=False,
                    )
                    nc.tensor.matmul(
                        ps[:, :], xt1[:, c0 : c0 + P], wkt[:, U : 2 * U],
                        start=False, stop=True,
                    )
                    nc.scalar.copy(hb[:, t * U : (t + 1) * U], ps[:, :])
                # pair layout: node n -> pair row n>>1, half n&1.
                # nodes n0..n0+ncnt-1 -> pair rows n0/2 .. n0/2+ncnt/2-1
                dr = h_pair[n0 // 2 : (n0 + ncnt) // 2, :].rearrange(
                    "(o r) su -> (r su) o", r=P // 2
                ).rearrange("(r s u) o -> (r s) o u", s=2, u=U)
                srcv = hb[:].rearrange("p (o u) -> p o u", u=U)[:, 0:ntiles, :]
                h_writes.append(nc.sync.dma_start(dr, srcv))

            # ---------------- phase 2: gather + median ----------------
            ctx1.close()
            g_net = ctx.enter_context(tc.tile_pool(name="g_net", bufs=net_bufs))
            g_srt = ctx.enter_context(tc.tile_pool(name="g_srt", bufs=2))
            g_idx = ctx.enter_context(tc.tile_pool(name="g_idx", bufs=2))
            g_out = ctx.enter_context(tc.tile_pool(name="g_out", bufs=2))

            nc.gpsimd.load_library(library_config.mlp)
            n_g = 0
            ioff = joff = ooff = 0
            # all per-chunk tiles are allocated at MEGA size and prefix-
            # sliced for smaller chunks, so every chunk shares one set of
            # double-buffered tags
            BM = MEGA // P
            for c, C in enumerate(grid):
                B = C // P
                BU = B * U
                NIDX = C * K
                IDXCOLS = NIDX // 16
                J = NIDX // P
                ia_f = g_idx.tile([P, MEGA * K // 16], I16, tag="ia")
                pa_f = g_idx.tile([P, MEGA * K // P], U8, tag="pa")
                ia = ia_f[:, 0:IDXCOLS]
                pa = pa_f[:, 0:J]
                nc.sync.dma_start(ia, idx[:, ioff : ioff + IDXCOLS])
                nc.sync.dma_start(pa, par[:, joff : joff + J])
                pt_f = g_net.tile([P, (MEGA * K // P) * 2 * U], F16, tag="pt")
                pt = pt_f[:, 0 : J * 2 * U]
                # gather calls of <=8192 256B pair rows each; chunk 0 is
                # split finer so parity resolution starts before the whole
                # first gather lands (shortens the phase-1 ramp)
                split = max(1, NIDX // MAX_GATHER)
                if c == 0:
                    split *= 2
                per = NIDX // split
                assert per % P == 0
                p4 = pt.rearrange("p (j s u) -> p j s u", s=2, u=U)
                for gsp in range(split):
                    jj0 = gsp * (per // P)
                    jj1 = (gsp + 1) * (per // P)
                    g = nc.gpsimd.dma_gather(
                        pt[:, jj0 * 2 * U : jj1 * 2 * U].rearrange(
                            "p (j e) -> p j e", e=2 * U
                        ),
                        h_pair[:],
                        ia[:, gsp * per // 16 : (gsp + 1) * per // 16],
                        per,
                        per,
                        2 * U,
                        single_packet=False,
                    )
                    if n_g == 0:
                        for w in h_writes:
                            add_dep_helper(
                                g.ins, w.ins,
                                reason="gather waits for h DRAM writes",
                            )
                    n_g += 1
                    # resolve pair parity in place: even half (s=0) is the
                    # message home; overwrite with odd half where parity=1
                    mask = pa[:, jj0:jj1].unsqueeze(2).broadcast_to(
                        [P, jj1 - jj0, U]
                    )
                    nc.vector.copy_predicated(
                        out=p4[:, jj0:jj1, 0, :], mask=mask,
                        data=p4[:, jj0:jj1, 1, :],
                    )

                ra_f = g_srt.tile([P, K * BM * U], F16, tag="ra")
                rb_f = g_srt.tile([P, K * BM * U], F16, tag="rb")
                ra = ra_f[:, 0 : K * BU]
                rb = rb_f[:, 0 : K * BU]

                # stage 1 of the Batcher network reads the strided message
                # view (s=0 halves of pt) and writes compact k-major planes
                msg = pt.rearrange(
                    "p (hi r b s u) -> p hi r b s u", hi=16, r=2, b=B, s=2, u=U
                )[:, :, :, :, 0, :]
                vd = ra.rearrange("p (hi r bu) -> p hi r bu", r=2, bu=BU)
                lo_s = msg[:, :, 0, :, :]
                hi_s = msg[:, :, 1, :, :]
                nc.vector.tensor_tensor(
                    out=vd[:, :, 0, :].rearrange("p hi (b u) -> p hi b u", u=U),
                    in0=lo_s, in1=hi_s, op=mybir.AluOpType.min,
                )
                nc.vector.tensor_tensor(
                    out=vd[:, :, 1, :].rearrange("p hi (b u) -> p hi b u", u=U),
                    in0=lo_s, in1=hi_s, op=mybir.AluOpType.max,
                )

                # Batcher network stages 2..10 over both halves, ping-pong
                src, dst = ra, rb
                for sp in SORT16_STAGES[1:]:
                    f = sp["f"]
                    ni = 16 // f
                    i_full = sp["i"] == (0, ni, 1)
                    d = sp["d"]
                    di, dr = d // f, d % f
                    r_vals = list(range(*sp["r"]))
                    if r_vals[-1] + dr >= f:
                        assert all(rv + dr >= f for rv in r_vals), sp
                        di, dr = di + 1, dr - f
                    r_sl = slice(*sp["r"])
                    hi_r = slice(sp["r"][0] + dr, sp["r"][1] + dr, sp["r"][2])
                    if i_full and di == 0:
                        vs = src.rearrange("p (hi r bu) -> p hi r bu", r=f, bu=BU)
                        vd = dst.rearrange("p (hi r bu) -> p hi r bu", r=f, bu=BU)
                        lo_s = vs[:, :, r_sl, :]
                        hi_s = vs[:, :, hi_r, :]
                        nc.vector.tensor_tensor(
                            out=vd[:, :, r_sl, :], in0=lo_s, in1=hi_s,
                            op=mybir.AluOpType.min,
                        )
                        nc.vector.tensor_tensor(
                            out=vd[:, :, hi_r, :], in0=lo_s, in1=hi_s,
                            op=mybir.AluOpType.max,
                        )
                    else:
                        i_sl = slice(*sp["i"])
                        hi_i = slice(sp["i"][0] + di, sp["i"][1] + di, sp["i"][2])
                        vs = src.rearrange(
                            "p (hh i r bu) -> p hh i r bu", hh=2, i=ni, r=f, bu=BU
                        )
                        vd = dst.rearrange(
                            "p (hh i r bu) -> p hh i r bu", hh=2, i=ni, r=f, bu=BU
                        )
                        lo_s = vs[:, :, i_sl, r_sl, :]
                        hi_s = vs[:, :, hi_i, hi_r, :]
                        nc.vector.tensor_tensor(
                            out=vd[:, :, i_sl, r_sl, :], in0=lo_s, in1=hi_s,
                            op=mybir.AluOpType.min,
                        )
                        nc.vector.tensor_tensor(
                            out=vd[:, :, hi_i, hi_r, :], in0=lo_s, in1=hi_s,
                            op=mybir.AluOpType.max,
                        )
                    vks = src.rearrange("p (hh kk bu) -> p hh kk bu", hh=2, kk=16)
                    vkd = dst.rearrange("p (hh kk bu) -> p hh kk bu", hh=2, kk=16)
                    for cpsl in sp["cp"]:
                        ks = slice(*cpsl)
                        nc.scalar.copy(vkd[:, :, ks, :], vks[:, :, ks, :])
                    src, dst = dst, src

                # anti-diagonal merge of the two sorted 16-plane halves
                # (min/max must run on DVE: the Pool ucode only implements
                # add/mult). Results go to a separate small tile so ra/rb
                # free up for the next chunk once the antidiag ops retire.
                vk = src.rearrange("p (k bu) -> p k bu", k=K)
                A = vk[:, 0:16, :]
                Brev = vk[:, 31:15:-1, :]
                tlu_f = g_out.tile([P, 2 * 16 * BM * U], F16, tag="tlu")
                vlu = tlu_f[:, 0 : 2 * 16 * BU].rearrange(
                    "p (s k bu) -> p s k bu", s=2, bu=BU
                )
                nc.vector.tensor_tensor(
                    out=vlu[:, 0, :, :], in0=A, in1=Brev, op=mybir.AluOpType.min
                )
                nc.vector.tensor_tensor(
                    out=vlu[:, 1, :, :], in0=A, in1=Brev, op=mybir.AluOpType.max
                )
                # Both reduction trees (max-of-mins for low in s=0,
                # min-of-maxes for up in s=1): 16 -> 8 -> 4 -> 2 -> 1.
                w = 16
                while w > 1:
                    h = w // 2
                    nc.vector.tensor_tensor(
                        out=vlu[:, 0, 0:h, :], in0=vlu[:, 0, 0:h, :],
                        in1=vlu[:, 0, h:w, :], op=mybir.AluOpType.max,
                    )
                    nc.vector.tensor_tensor(
                        out=vlu[:, 1, 0:h, :], in0=vlu[:, 1, 0:h, :],
                        in1=vlu[:, 1, h:w, :], op=mybir.AluOpType.min,
                    )
                    w = h
                med_f = g_out.tile([P, BM * U], F32, tag="med")
                med = med_f[:, 0:BU]
                nc.vector.tensor_tensor(
                    out=med, in0=vlu[:, 0, 0, :], in1=vlu[:, 1, 0, :],
                    op=mybir.AluOpType.add,
                )
                nc.sync.dma_start(out[:, ooff : ooff + BU], med)
                ioff += IDXCOLS
                joff += J
                ooff += BU

    nc.compile()
    return nc


def _prep_inputs(x, neighbors, kern, num_cores=NUM_CORES):
    nrows = x.shape[0]
    total = neighbors.shape[0]
    shard = (total + num_cores - 1) // num_cores
    grid = _chunk_grid(shard)
    shard_pad = sum(grid)
    IDXTOT = sum(Cc * K // 16 for Cc in grid)
    JTOT = sum(Cc * K // P for Cc in grid)

    nrows_pad = ((nrows + P - 1) // P) * P
    xT = np.zeros((FEAT, nrows_pad), dtype=np.float16)
    xT[:, :nrows] = x.T
    # fold the midpoint *0.5 into the weights (median is scale-equivariant)
    wk = np.ascontiguousarray(kern * 0.5).astype(np.float16)

    in_maps = []
    for core in range(num_cores):
        n0 = core * shard
        nbr = np.zeros((shard_pad, K), dtype=np.int64)
        real = min(shard, total - n0)
        nbr[:real] = neighbors[n0 : n0 + real]
        idxarr = np.empty((P, IDXTOT), dtype=np.int16)
        pararr = np.empty((P, JTOT), dtype=np.uint8)
        noff = ioff = joff = 0
        for Cc in grid:
            B = Cc // P
            NIDX = Cc * K
            IDXCOLS = NIDX // 16
            J = NIDX // P
            # i = ((k*B + b)*128 + p) enumerates (plane k, block b, part p)
            nb = (
                nbr[noff : noff + Cc]
                .reshape(B, P, K)
                .transpose(2, 0, 1)
                .reshape(NIDX)
            )
            pairs = (nb >> 1).astype(np.int16)
            parity = (nb & 1).astype(np.uint8)
            # logical index i lives at [i%16, i//16]; replicated to all
            # eight 16-partition groups (Q7 core pairs read their own)
            idxarr[:, ioff : ioff + IDXCOLS] = np.tile(
                pairs.reshape(IDXCOLS, 16).T, (P // 16, 1)
            )
            pararr[:, joff : joff + J] = parity.reshape(J, P).T
            noff += Cc
            ioff += IDXCOLS
            joff += J
        in_maps.append({"xT": xT, "wk": wk, "idx": idxarr, "par": pararr})
    meta = dict(shard=shard, shard_pad=shard_pad, grid=grid, total=total)
    return in_maps, meta


def _unshard_output(results, meta, num_cores=NUM_CORES):
    outs = []
    for core in range(num_cores):
        o = results[core]["out"]  # [P, BUTOT]
        parts = []
        ooff = 0
        for Cc in meta["grid"]:
            B = Cc // P
            BU = B * U
            blk = o[:, ooff : ooff + BU].reshape(P, B, U).transpose(1, 0, 2)
            parts.append(blk.reshape(Cc, U))
            ooff += BU
        outs.append(np.concatenate(parts, axis=0)[: meta["shard"]])
    return np.concatenate(outs, axis=0)[: meta["total"]]


_CACHE = {}


def kernel(x, neighbors, kernel):
    """Full inputs in, full output out. Shards nodes across 8 NeuronCores."""
    x = np.asarray(x, dtype=np.float32)
    neighbors_np = np.asarray(neighbors)
    kern = np.asarray(kernel, dtype=np.float32)
    assert x.shape[1] == FEAT and kern.shape == (FEAT, U)
    assert neighbors_np.shape[1] == K

    in_maps, meta = _prep_inputs(x, neighbors_np, kern)
    key = (x.shape[0], meta["shard"])
    if key not in _CACHE:
        _CACHE[key] = build_kernel(x.shape[0], meta["shard"])
    nc = _CACHE[key]
    res = bass_utils.run_bass_kernel_spmd(
        nc, in_maps, core_ids=list(range(NUM_CORES))
    )
    return _unshard_output(res.results, meta)


# revision 26
# speedup vs baseline: 1.8809x; 1.0025x over previous
"""MedianConvolution (gnn message passing) — Trainium2 Bass kernel, 8 cores.

Computes: h = x @ kernel; msg = h[neighbors]; out = exact midpoint median
over the K=32 neighbor axis (ranks 15,16 of the sort), i.e.
tfp percentile(q=50, interpolation='midpoint').

Distribution: nodes (rows of neighbors) are sharded across the 8 NeuronCores;
every core computes the full h = x @ kernel on-device (x/kernel replicated,
fp16 inputs, fp32 PSUM accumulate) and gathers only its own node shard's
neighbor rows.

Key layout trick: h is stored in DRAM as PAIRS h_pair[r] = [h[2r] | h[2r+1]]
(fp16, 256B rows). dma_gather requires 256B-aligned elements and int16
indices; the pair layout satisfies both (idx = node>>1 <= 24999) with ONE
descriptor per neighbor instead of the two (lo/hi halves) an fp32 layout
needs. The wrong pair half is discarded on-chip with a single
copy_predicated keyed on a host-uploaded parity mask.

The whole median datapath runs in fp16: TensorTensor min/max supports the
DVE 2x_1p fast mode (2-byte dtypes) for 2x throughput, and the final
midpoint is exact up to fp16 rounding (~0.05%), far inside the 2e-2 gate.
The x0.5 of the midpoint is folded into the GEMM weights (median is
scale-equivariant for positive scales). min/max only exist on the DVE
(the GPSIMD ucode implements only add/mult), so the whole median pipeline
lives there; plane copies go to the Scalar engine, descriptor generation
to GPSIMD, and the node shard is processed in 512-node mega-chunks (after
a 128-node head chunk that fills the pipeline quickly, plus a padded tail)
to amortize per-instruction overheads on the bottleneck DVE.

Per-core SPMD program:
  phase 1  GEMM: xT fp16 [256, N] x wk fp16 [256, 64] -> PSUM fp32, copied
           to fp16 and DMAed into the pair layout.
  phase 2  per chunk of C shard nodes: dma_gather pulls the 256B pair rows
           for all 32 neighbor planes (k-major, <=8192 indices per call), a
           copy_predicated resolves pair parity in place, a Batcher
           odd-even mergesort sorts planes 0-15 and 16-31 (fp16 TT min/max,
           untouched planes copied on the Scalar engine), the 32-way median
           pair comes from the anti-diagonal identity
              low = max_i min(A_i, B_15-i),  up = min_i max(A_i, B_15-i)
           via two TT ops + two min/max trees, and low+up (already scaled
           by 0.5) is written out in fp32.
"""
from contextlib import ExitStack

import numpy as np

import concourse.bass as bass
import concourse.tile as tile
from concourse import bacc, bass_utils, library_config, mybir
from concourse.tile_rust import add_dep_helper

F32 = mybir.dt.float32
F16 = mybir.dt.float16
I16 = mybir.dt.int16
U8 = mybir.dt.uint8
P = 128
U = 64  # units
K = 32  # neighbors
FEAT = 256
N_NODES = 50000
NUM_CORES = 8
MEGA = 512        # main chunk size (nodes); tail chunk is the padded rest
NET_BUFS = 2
MAX_GATHER = 8192  # max indices per dma_gather call (HW-validated)

# Batcher odd-even mergesort(16) stages; verified against np.sort via the
# 0-1 principle. Each stage: comparators (k, k+d) for k = i*f + r over the
# slices below, applied to both 16-plane halves. cp = untouched plane
# slices (copied to the ping-pong destination).
SORT16_STAGES = [
    dict(f=2, i=(0, 8, 1), r=(0, 1, 1), d=1, cp=[]),
    dict(f=4, i=(0, 4, 1), r=(0, 2, 1), d=2, cp=[]),
    dict(f=4, i=(0, 4, 1), r=(1, 2, 1), d=1, cp=[(0, 16, 4), (3, 16, 4)]),
    dict(f=8, i=(0, 2, 1), r=(0, 4, 1), d=4, cp=[]),
    dict(f=8, i=(0, 2, 1), r=(2, 4, 1), d=2,
         cp=[(0, 16, 8), (1, 16, 8), (6, 16, 8), (7, 16, 8)]),
    dict(f=8, i=(0, 2, 1), r=(1, 6, 2), d=1, cp=[(0, 16, 8), (7, 16, 8)]),
    dict(f=16, i=(0, 1, 1), r=(0, 8, 1), d=8, cp=[]),
    dict(f=16, i=(0, 1, 1), r=(4, 8, 1), d=4, cp=[(0, 4, 1), (12, 16, 1)]),
    dict(f=4, i=(0, 3, 1), r=(2, 4, 1), d=2, cp=[(0, 2, 1), (14, 16, 1)]),
    dict(f=2, i=(0, 7, 1), r=(1, 2, 1), d=1, cp=[(0, 16, 15)]),
]


def _chunk_grid(shard):
    """Chunk sizes covering `shard` nodes: a small first chunk (so the
    gather->sort pipeline fills quickly after the GEMM), MEGA-chunks for
    the bulk, and a padded tail."""
    grid = []
    rest = shard
    if rest > P + MEGA:
        grid.append(P)
        rest -= P
    grid += [MEGA] * (rest // MEGA)
    rem = rest - (rest // MEGA) * MEGA
    if rem:
        grid.append(((rem + P - 1) // P) * P)
    return grid


def build_kernel(nrows, shard, num_cores=NUM_CORES, gemm_super=2048,
                 net_bufs=NET_BUFS):
    nrows = ((nrows + P - 1) // P) * P  # host pads xT to the same size
    NPAIR = nrows // 2
    grid = _chunk_grid(shard)
    IDXTOT = sum(Cc * K // 16 for Cc in grid)
    JTOT = sum(Cc * K // P for Cc in grid)
    BUTOT = sum(Cc // 2 for Cc in grid)

    nc = bacc.Bacc(
        "TRN2",
        target_bir_lowering=False,
        debug=False,
        num_devices=num_cores,
    )

    xT = nc.dram_tensor("xT", [FEAT, nrows], F16, kind="ExternalInput").ap()
    wk = nc.dram_tensor("wk", [FEAT, U], F16, kind="ExternalInput").ap()
    idx = nc.dram_tensor("idx", [P, IDXTOT], I16, kind="ExternalInput").ap()
    par = nc.dram_tensor("par", [P, JTOT], U8, kind="ExternalInput").ap()
    out = nc.dram_tensor("out", [P, BUTOT], F32, kind="ExternalOutput").ap()
    h_pair = nc.dram_tensor("h_pair", [NPAIR, 2 * U], F16, kind="Internal").ap()

    with tile.TileContext(nc) as tc:
        with ExitStack() as ctx:
            # ---------------- phase 1: GEMM ----------------
            ctx1 = ctx.enter_context(ExitStack())
            g_x = ctx1.enter_context(tc.tile_pool(name="g_x", bufs=3))
            g_w = ctx1.enter_context(tc.tile_pool(name="g_w", bufs=1))
            g_h = ctx1.enter_context(tc.tile_pool(name="g_h", bufs=3))
            g_ps = ctx1.enter_context(tc.tile_pool(name="g_ps", bufs=8, space="PSUM"))

            wkt = g_w.tile([P, 2 * U], F16)
            nc.sync.dma_start(wkt[:, 0:U], wk[0:P, :])
            nc.sync.dma_start(wkt[:, U : 2 * U], wk[P : 2 * P, :])

            h_writes = []
            S = gemm_super
            n_super = (nrows + S - 1) // S
            for s in range(n_super):
                n0 = s * S
                ncnt = min(S, nrows - n0)
                assert ncnt % P == 0
                ntiles = ncnt // P
                xt0 = g_x.tile([P, S], F16, tag="xt0")
                xt1 = g_x.tile([P, S], F16, tag="xt1")
                nc.sync.dma_start(xt0[:, 0:ncnt], xT[0:P, n0 : n0 + ncnt])
                nc.sync.dma_start(xt1[:, 0:ncnt], xT[P : 2 * P, n0 : n0 + ncnt])
                hb = g_h.tile([P, (S // P) * U], F16, tag="hb")
                for t in range(ntiles):
                    c0 = t * P
                    ps = g_ps.tile([P, U], F32)
                    nc.tensor.matmul(
                        ps[:, :], xt0[:, c0 : c0 + P], wkt[:, 0:U],
                        start=True, stop=False,
                    )
                    nc.tensor.matmul(
                        ps[:, :], xt1[:, c0 : c0 + P], wkt[:, U : 2 * U],
                        start=False, stop=True,
                    )
                    nc.scalar.copy(hb[:, t * U : (t + 1) * U], ps[:, :])
                # pair layout: node n -> pair row n>>1, half n&1.
                # nodes n0..n0+ncnt-1 -> pair rows n0/2 .. n0/2+ncnt/2-1
                dr = h_pair[n0 // 2 : (n0 + ncnt) // 2, :].rearrange(
                    "(o r) su -> (r su) o", r=P // 2
                ).rearrange("(r s u) o -> (r s) o u", s=2, u=U)
                srcv = hb[:].rearrange("p (o u) -> p o u", u=U)[:, 0:ntiles, :]
                h_writes.append(nc.sync.dma_start(dr, srcv))

            # ---------------- phase 2: gather + median ----------------
            ctx1.close()
            g_net = ctx.enter_context(tc.tile_pool(name="g_net", bufs=net_bufs))
            g_srt = ctx.enter_context(tc.tile_pool(name="g_srt", bufs=2))
            g_idx = ctx.enter_context(tc.tile_pool(name="g_idx", bufs=2))
            g_out = ctx.enter_context(tc.tile_pool(name="g_out", bufs=2))

            nc.gpsimd.load_library(library_config.mlp)
            n_g = 0
            ioff = joff = ooff = 0
            # all per-chunk tiles are allocated at MEGA size and prefix-
            # sliced for smaller chunks, so every chunk shares one set of
            # double-buffered tags
            BM = MEGA // P
            for c, C in enumerate(grid):
                B = C // P
                BU = B * U
                NIDX = C * K
                IDXCOLS = NIDX // 16
                J = NIDX // P
                ia_f = g_idx.tile([P, MEGA * K // 16], I16, tag="ia")
                pa_f = g_idx.tile([P, MEGA * K // P], U8, tag="pa")
                ia = ia_f[:, 0:IDXCOLS]
                pa = pa_f[:, 0:J]
                nc.sync.dma_start(ia, idx[:, ioff : ioff + IDXCOLS])
                nc.sync.dma_start(pa, par[:, joff : joff + J])
                pt_f = g_net.tile([P, (MEGA * K // P) * 2 * U], F16, tag="pt")
                pt = pt_f[:, 0 : J * 2 * U]
                # gather calls of <=8192 256B pair rows each; chunk 0 is
                # split finer so parity resolution starts before the whole
                # first gather lands (shortens the phase-1 ramp)
                split = max(1, NIDX // MAX_GATHER)
                if c == 0:
                    split *= 2
                per = NIDX // split
                assert per % P == 0
                p4 = pt.rearrange("p (j s u) -> p j s u", s=2, u=U)
                for gsp in range(split):
                    jj0 = gsp * (per // P)
                    jj1 = (gsp + 1) * (per // P)
                    g = nc.gpsimd.dma_gather(
                        pt[:, jj0 * 2 * U : jj1 * 2 * U].rearrange(
                            "p (j e) -> p j e", e=2 * U
                        ),
                        h_pair[:],
                        ia[:, gsp * per // 16 : (gsp + 1) * per // 16],
                        per,
                        per,
                        2 * U,
                        single_packet=False,
                    )
                    if n_g == 0:
                        for w in h_writes:
                            add_dep_helper(
                                g.ins, w.ins,
                                reason="gather waits for h DRAM writes",
                            )
                    n_g += 1
                    # resolve pair parity in place: even half (s=0) is the
                    # message home; overwrite with odd half where parity=1
                    mask = pa[:, jj0:jj1].unsqueeze(2).broadcast_to(
                        [P, jj1 - jj0, U]
                    )
                    nc.vector.copy_predicated(
                        out=p4[:, jj0:jj1, 0, :], mask=mask,
                        data=p4[:, jj0:jj1, 1, :],
                    )

                ra_f = g_srt.tile([P, K * BM * U], F16, tag="ra")
                rb_f = g_srt.tile([P, K * BM * U], F16, tag="rb")
                ra = ra_f[:, 0 : K * BU]
                rb = rb_f[:, 0 : K * BU]

                # stage 1 of the Batcher network reads the strided message
                # view (s=0 halves of pt) and writes compact k-major planes
                msg = pt.rearrange(
                    "p (hi r b s u) -> p hi r b s u", hi=16, r=2, b=B, s=2, u=U
                )[:, :, :, :, 0, :]
                vd = ra.rearrange("p (hi r bu) -> p hi r bu", r=2, bu=BU)
                lo_s = msg[:, :, 0, :, :]
                hi_s = msg[:, :, 1, :, :]
                nc.vector.tensor_tensor(
                    out=vd[:, :, 0, :].rearrange("p hi (b u) -> p hi b u", u=U),
                    in0=lo_s, in1=hi_s, op=mybir.AluOpType.min,
                )
                nc.vector.tensor_tensor(
                    out=vd[:, :, 1, :].rearrange("p hi (b u) -> p hi b u", u=U),
                    in0=lo_s, in1=hi_s, op=mybir.AluOpType.max,
                )

                # Batcher network stages 2..10 over both halves, ping-pong
                src, dst = ra, rb
                for sp in SORT16_STAGES[1:]:
                    f = sp["f"]
                    ni = 16 // f
                    i_full = sp["i"] == (0, ni, 1)
                    d = sp["d"]
                    di, dr = d // f, d % f
                    r_vals = list(range(*sp["r"]))
                    if r_vals[-1] + dr >= f:
                        assert all(rv + dr >= f for rv in r_vals), sp
                        di, dr = di + 1, dr - f
                    r_sl = slice(*sp["r"])
                    hi_r = slice(sp["r"][0] + dr, sp["r"][1] + dr, sp["r"][2])
                    if i_full and di == 0:
                        vs = src.rearrange("p (hi r bu) -> p hi r bu", r=f, bu=BU)
                        vd = dst.rearrange("p (hi r bu) -> p hi r bu", r=f, bu=BU)
                        lo_s = vs[:, :, r_sl, :]
                        hi_s = vs[:, :, hi_r, :]
                        nc.vector.tensor_tensor(
                            out=vd[:, :, r_sl, :], in0=lo_s, in1=hi_s,
                            op=mybir.AluOpType.min,
                        )
                        nc.vector.tensor_tensor(
                            out=vd[:, :, hi_r, :], in0=lo_s, in1=hi_s,
                            op=mybir.AluOpType.max,
                        )
                    else:
                        i_sl = slice(*sp["i"])
                        hi_i = slice(sp["i"][0] + di, sp["i"][1] + di, sp["i"][2])
                        vs = src.rearrange(
                            "p (hh i r bu) -> p hh i r bu", hh=2, i=ni, r=f, bu=BU
                        )
                        vd = dst.rearrange(
                            "p (hh i r bu) -> p hh i r bu", hh=2, i=ni, r=f, bu=BU
                        )
                        lo_s = vs[:, :, i_sl, r_sl, :]
                        hi_s = vs[:, :, hi_i, hi_r, :]
                        nc.vector.tensor_tensor(
                            out=vd[:, :, i_sl, r_sl, :], in0=lo_s, in1=hi_s,
                            op=mybir.AluOpType.min,
                        )
                        nc.vector.tensor_tensor(
                            out=vd[:, :, hi_i, hi_r, :], in0=lo_s, in1=hi_s,
                            op=mybir.AluOpType.max,
                        )
                    vks = src.rearrange("p (hh kk bu) -> p hh kk bu", hh=2, kk=16)
                    vkd = dst.rearrange("p (hh kk bu) -> p hh kk bu", hh=2, kk=16)
                    for cpsl in sp["cp"]:
                        ks = slice(*cpsl)
                        nc.scalar.copy(vkd[:, :, ks, :], vks[:, :, ks, :])
                    src, dst = dst, src

                # anti-diagonal merge of the two sorted 16-plane halves
                # (min/max must run on DVE: the Pool ucode only implements
                # add/mult). Results go to a separate small tile so ra/rb
                # free up for the next chunk once the antidiag ops retire.
                vk = src.rearrange("p (k bu) -> p k bu", k=K)
                A = vk[:, 0:16, :]
                Brev = vk[:, 31:15:-1, :]
                tlu_f = g_out.tile([P, 2 * 16 * BM * U], F16, tag="tlu")
                vlu = tlu_f[:, 0 : 2 * 16 * BU].rearrange(
                    "p (s k bu) -> p s k bu", s=2, bu=BU
                )
                nc.vector.tensor_tensor(
                    out=vlu[:, 0, :, :], in0=A, in1=Brev, op=mybir.AluOpType.min
                )
                nc.vector.tensor_tensor(
                    out=vlu[:, 1, :, :], in0=A, in1=Brev, op=mybir.AluOpType.max
                )
                # Both reduction trees (max-of-mins for low in s=0,
                # min-of-maxes for up in s=1): 16 -> 8 -> 4 -> 2 -> 1.
                w = 16
                while w > 1:
                    h = w // 2
                    nc.vector.tensor_tensor(
                        out=vlu[:, 0, 0:h, :], in0=vlu[:, 0, 0:h, :],
                        in1=vlu[:, 0, h:w, :], op=mybir.AluOpType.max,
                    )
                    nc.vector.tensor_tensor(
                        out=vlu[:, 1, 0:h, :], in0=vlu[:, 1, 0:h, :],
                        in1=vlu[:, 1, h:w, :], op=mybir.AluOpType.min,
                    )
                    w = h
                med_f = g_out.tile([P, BM * U], F32, tag="med")
                med = med_f[:, 0:BU]
                nc.vector.tensor_tensor(
                    out=med, in0=vlu[:, 0, 0, :], in1=vlu[:, 1, 0, :],
                    op=mybir.AluOpType.add,
                )
                nc.sync.dma_start(out[:, ooff : ooff + BU], med)
                ioff += IDXCOLS
                joff += J
                ooff += BU

    nc.compile()
    return nc


def _prep_inputs(x, neighbors, kern, num_cores=NUM_CORES):
    nrows = x.shape[0]
    total = neighbors.shape[0]
    shard = (total + num_cores - 1) // num_cores
    grid = _chunk_grid(shard)
    shard_pad = sum(grid)
    IDXTOT = sum(Cc * K // 16 for Cc in grid)
    JTOT = sum(Cc * K // P for Cc in grid)

    nrows_pad = ((nrows + P - 1) // P) * P
    xT = np.zeros((FEAT, nrows_pad), dtype=np.float16)
    xT[:, :nrows] = x.T
    # fold the midpoint *0.5 into the weights (median is scale-equivariant)
    wk = np.ascontiguousarray(kern * 0.5).astype(np.float16)

    in_maps = []
    for core in range(num_cores):
        n0 = core * shard
        nbr = np.zeros((shard_pad, K), dtype=np.int64)
        real = min(shard, total - n0)
        nbr[:real] = neighbors[n0 : n0 + real]
        idxarr = np.empty((P, IDXTOT), dtype=np.int16)
        pararr = np.empty((P, JTOT), dtype=np.uint8)
        noff = ioff = joff = 0
        for Cc in grid:
            B = Cc // P
            NIDX = Cc * K
            IDXCOLS = NIDX // 16
            J = NIDX // P
            # i = ((k*B + b)*128 + p) enumerates (plane k, block b, part p)
            nb = (
                nbr[noff : noff + Cc]
                .reshape(B, P, K)
                .transpose(2, 0, 1)
                .reshape(NIDX)
            )
            pairs = (nb >> 1).astype(np.int16)
            parity = (nb & 1).astype(np.uint8)
            # logical index i lives at [i%16, i//16]; replicated to all
            # eight 16-partition groups (Q7 core pairs read their own)
            idxarr[:, ioff : ioff + IDXCOLS] = np.tile(
                pairs.reshape(IDXCOLS, 16).T, (P // 16, 1)
            )
            pararr[:, joff : joff + J] = parity.reshape(J, P).T
            noff += Cc
            ioff += IDXCOLS
            joff += J
        in_maps.append({"xT": xT, "wk": wk, "idx": idxarr, "par": pararr})
    meta = dict(shard=shard, shard_pad=shard_pad, grid=grid, total=total)
    return in_maps, meta


def _unshard_output(results, meta, num_cores=NUM_CORES):
    outs = []
    for core in range(num_cores):
        o = results[core]["out"]  # [P, BUTOT]
        parts = []
        ooff = 0
        for Cc in meta["grid"]:
            B = Cc // P
            BU = B * U
            blk = o[:, ooff : ooff + BU].reshape(P, B, U).transpose(1, 0, 2)
            parts.append(blk.reshape(Cc, U))
            ooff += BU
        outs.append(np.concatenate(parts, axis=0)[: meta["shard"]])
    return np.concatenate(outs, axis=0)[: meta["total"]]


_CACHE = {}


def kernel(x, neighbors, kernel):
    """Full inputs in, full output out. Shards nodes across 8 NeuronCores."""
    x = np.asarray(x, dtype=np.float32)
    neighbors_np = np.asarray(neighbors)
    kern = np.asarray(kernel, dtype=np.float32)
    assert x.shape[1] == FEAT and kern.shape == (FEAT, U)
    assert neighbors_np.shape[1] == K

    in_maps, meta = _prep_inputs(x, neighbors_np, kern)
    key = (x.shape[0], meta["shard"])
    if key not in _CACHE:
        _CACHE[key] = build_kernel(x.shape[0], meta["shard"])
    nc = _CACHE[key]
    res = bass_utils.run_bass_kernel_spmd(
        nc, in_maps, core_ids=list(range(NUM_CORES))
    )
    return _unshard_output(res.results, meta)
